# revision 9
# baseline (speedup 1.0000x reference)
"""Trainium2 Bass kernel for nn_Attention_7954279432380.

Reference math (B=8, L=4096, D=512, h=16, n=L//h=256):
    qr = q.reshape(B, h, n, D); k = ones(h, n, D); vr = v.reshape(B, h, n, D)
    scores[b,n,h,l] = sum_d qr[b,h,n,d] * k[h,l,d]   -> independent of l
    A = softmax(scale*scores, axis=-1)               -> exactly 1/n everywhere
    V[b,n,h,d] = A[b,n,h,n] * sum_l vr[b,l,n,d]      -> (1/n) * sum_l vr[b,l,n,d]

So A is the constant 1/256 (exact in f32: softmax of equal values is
exp(0)/n = 1/n) and V is a 16-way chunk-sum of v scaled by 1/256,
broadcast 16x along the head axis.  The kernel computes exactly that:
data-parallel over batch, one NeuronCore per batch element.
"""

import os
import sys

import numpy as np

for _p in ("/opt/trn_rl_repo", "/root/.axon_site/_ro/trn_rl_repo"):
    if os.path.isdir(_p) and _p not in sys.path:
        sys.path.append(_p)

B, L, D = 8, 4096, 512
H = 16
N = L // H  # 256 (= softmax length l, so both scale factors are 1/256)
NCORES = 8
RECIP = 1.0 / float(N)

_CACHE = {}


def _build_nc(krep=1):
    import concourse.tile as tile
    from concourse import bacc, mybir

    f32 = mybir.dt.float32
    nc = bacc.Bacc("TRN2", target_bir_lowering=False, debug=False,
                   num_devices=NCORES)
    # v shard for batch b, viewed as (l, g, p, d): row l*256 + g*128 + p of v[b]
    v = nc.dram_tensor("v", [H, 2, 128, D], f32, kind="ExternalInput").ap()
    # out_v viewed as (g, p, h, d): row 16*(g*128+p) + h of V[b]
    out_v = nc.dram_tensor("out_v", [2, 128, H, D], f32,
                           kind="ExternalOutput").ap()
    # out_a: flat 4MB of the constant 1/256 -> host reshapes to (256,16,256)
    out_a = nc.dram_tensor("out_a", [4, 128, 2048], f32,
                           kind="ExternalOutput").ap()

    with tile.TileContext(nc) as tc:
        with tc.tile_pool(name="vin", bufs=4) as vin, \
             tc.tile_pool(name="work", bufs=2) as work, \
             tc.tile_pool(name="aconst", bufs=1) as apool:
            atile = apool.tile([128, 2048], f32)
            nc.vector.memset(atile[:], RECIP)
            for rep in range(krep):
                # All 4 input loads (2MB each) first, on the sync/SP HWDGE
                # ring, so the DMA engines never starve.
                loads = []
                for g in range(2):
                    for j in range(2):
                        t = vin.tile([128, 8 * D], f32, tag="vload")
                        nc.sync.dma_start(
                            t[:].rearrange("p (l d) -> p l d", l=8),
                            v[8 * j:8 * j + 8, g, :, :].rearrange(
                                "l p d -> p l d"),
                        )
                        loads.append(t)
                # One 4MB constant-A write via a stride-0 source AP
                # (scalar/ACT HWDGE ring, so writes don't block loads).
                a_src = atile[:].unsqueeze(1).to_broadcast((128, 4, 2048))
                nc.scalar.dma_start(out_a[:].rearrange("k p c -> p k c"),
                                    a_src)

                for g in range(2):
                    t0, t1 = loads[2 * g], loads[2 * g + 1]
                    s = work.tile([128, 8 * D], f32, tag="s")
                    nc.vector.tensor_add(s[:], t0[:], t1[:])
                    r4 = work.tile([128, 4 * D], f32, tag="r4")
                    nc.vector.tensor_add(r4[:], s[:, 0:4 * D], s[:, 4 * D:8 * D])
                    r2 = work.tile([128, 2 * D], f32, tag="r2")
                    nc.vector.tensor_add(r2[:], r4[:, 0:2 * D], r4[:, 2 * D:4 * D])
                    r1 = work.tile([128, D], f32, tag="r1")
                    nc.vector.tensor_add(r1[:], r2[:, 0:D], r2[:, D:2 * D])
                    w = work.tile([128, D], f32, tag="w")
                    nc.vector.tensor_scalar_mul(w[:], r1[:], RECIP)
                    # One 4MB V write per half: 16-way head broadcast via
                    # stride-0 source dim -> fully contiguous DRAM write.
                    w_src = w[:].unsqueeze(1).to_broadcast((128, H, D))
                    nc.scalar.dma_start(out_v[g], w_src)

    nc.compile()
    return nc


def _get_nc():
    if "nc" not in _CACHE:
        _CACHE["nc"] = _build_nc()
    return _CACHE["nc"]


def _run(v_full, trace=False):
    from concourse.bass_utils import run_bass_kernel_spmd

    nc = _get_nc()
    in_maps = [
        {"v": np.ascontiguousarray(
            v_full[b].reshape(H, 2, 128, D), dtype=np.float32)}
        for b in range(NCORES)
    ]
    try:
        res = run_bass_kernel_spmd(nc, in_maps, list(range(NCORES)),
                                   trace=trace)
    except Exception:
        # One retry for transient execution-path failures.
        import time as _time
        _time.sleep(2.0)
        res = run_bass_kernel_spmd(nc, in_maps, list(range(NCORES)),
                                   trace=trace)
    V = np.empty((B, L, D), dtype=np.float32)
    A = np.empty((B, N, H, N), dtype=np.float32)
    for b in range(NCORES):
        V[b] = res.results[b]["out_v"].reshape(L, D)
        A[b] = res.results[b]["out_a"].reshape(N, H, N)
    return (V, A), res


def kernel(q, v):
    v = np.asarray(v, dtype=np.float32)
    (V, A), _ = _run(v, trace=False)
    return V, A


def kernel_profiled(q, v):
    v = np.asarray(v, dtype=np.float32)
    (V, A), res = _run(v, trace=False)
    return (V, A), res


# revision 10
# speedup vs baseline: 1.0043x; 1.0043x over previous
"""Trainium2 Bass kernel for nn_Attention_7954279432380.

Reference math (B=8, L=4096, D=512, h=16, n=L//h=256):
    qr = q.reshape(B, h, n, D); k = ones(h, n, D); vr = v.reshape(B, h, n, D)
    scores[b,n,h,l] = sum_d qr[b,h,n,d] * k[h,l,d]   -> independent of l
    A = softmax(scale*scores, axis=-1)               -> exactly 1/n everywhere
    V[b,n,h,d] = A[b,n,h,n] * sum_l vr[b,l,n,d]      -> (1/n) * sum_l vr[b,l,n,d]

So A is the constant 1/256 (exact in f32: softmax of equal values is
exp(0)/n = 1/n) and V is a 16-way chunk-sum of v scaled by 1/256,
broadcast 16x along the head axis.  The kernel computes exactly that:
data-parallel over batch, one NeuronCore per batch element; per core
8MB of reads + 12MB of writes, streamed gap-free at the HBM limit.

Default implementation is raw-bacc (explicit semaphores, minimal kernel
tail); set KERNEL_IMPL=tile for the TileContext-scheduled equivalent
(bitwise-identical outputs).  Raw semaphore protocol is race-free by a
totality argument: every wait_ge(sem, N) uses N = the sem's total
eventual value, so it passes only when every SDMA-lane increment of
every contributing DMA has fired.
"""

import os
import sys

import numpy as np

for _p in ("/opt/trn_rl_repo", "/root/.axon_site/_ro/trn_rl_repo"):
    if os.path.isdir(_p) and _p not in sys.path:
        sys.path.append(_p)

B, L, D = 8, 4096, 512
H = 16
N = L // H  # 256 (= softmax length l, so both scale factors are 1/256)
NCORES = 8
RECIP = 1.0 / float(N)

_CACHE = {}


def _build_raw(krep=1):
    """Raw-bacc build: no TileContext entry/exit barriers beyond the
    runtime-mandated init; all SBUF tiles are distinct allocations so the
    only hazards are RAW, closed by total-valued semaphore waits."""
    from concourse import bacc, mybir

    f32 = mybir.dt.float32
    nc = bacc.Bacc("TRN2", target_bir_lowering=False, debug=False,
                   num_devices=NCORES)
    v = nc.dram_tensor("v", [H, 2, 128, D], f32, kind="ExternalInput").ap()
    out_v = nc.dram_tensor("out_v", [2, 128, H, D], f32,
                           kind="ExternalOutput").ap()
    out_a = nc.dram_tensor("out_a", [4, 128, 2048], f32,
                           kind="ExternalOutput").ap()

    atile = nc.alloc_sbuf_tensor("atile", [128, 2048], f32).ap()
    lt = [[nc.alloc_sbuf_tensor(f"ld_{g}_{j}", [128, 8 * D], f32).ap()
           for j in range(2)] for g in range(2)]
    s = [nc.alloc_sbuf_tensor(f"s_{g}", [128, 8 * D], f32).ap()
         for g in range(2)]
    r4 = [nc.alloc_sbuf_tensor(f"r4_{g}", [128, 4 * D], f32).ap()
          for g in range(2)]
    r2 = [nc.alloc_sbuf_tensor(f"r2_{g}", [128, 2 * D], f32).ap()
          for g in range(2)]
    r1 = [nc.alloc_sbuf_tensor(f"r1_{g}", [128, D], f32).ap()
          for g in range(2)]
    w = [nc.alloc_sbuf_tensor(f"w_{g}", [128, D], f32).ap()
         for g in range(2)]

    with (
        nc.Block() as block,
        nc.semaphore("ld0") as ld0,
        nc.semaphore("ld1") as ld1,
        nc.semaphore("ms") as ms,
        nc.semaphore("ws") as ws,
        nc.semaphore("st") as st,
    ):
        lsem = [ld0, ld1]

        @block.sync
        def _(sync):
            for g in range(2):
                for j in range(2):
                    sync.dma_start(
                        out=lt[g][j][:].rearrange("p (l d) -> p l d", l=8),
                        in_=v[8 * j:8 * j + 8, g, :, :].rearrange(
                            "l p d -> p l d"),
                    ).then_inc(lsem[g], 16)
            sync.wait_ge(ld0, 32)
            sync.wait_ge(ld1, 32)

        @block.vector
        def _(vector):
            vector.memset(atile[:], RECIP).then_inc(ms)
            for g in range(2):
                vector.wait_ge(lsem[g], 32)
                vector.tensor_add(s[g][:], lt[g][0][:], lt[g][1][:])
                vector.tensor_add(r4[g][:], s[g][:, 0:4 * D],
                                  s[g][:, 4 * D:8 * D])
                vector.tensor_add(r2[g][:], r4[g][:, 0:2 * D],
                                  r4[g][:, 2 * D:4 * D])
                vector.tensor_add(r1[g][:], r2[g][:, 0:D], r2[g][:, D:2 * D])
                vector.tensor_scalar_mul(w[g][:], r1[g][:],
                                         RECIP).then_inc(ws)

        @block.scalar
        def _(scalar):
            scalar.wait_ge(ms, 1)
            a_src = atile[:].unsqueeze(1).to_broadcast((128, 4, 2048))
            scalar.dma_start(out=out_a[:].rearrange("k p c -> p k c"),
                             in_=a_src).then_inc(st, 16)
            for g in range(2):
                scalar.wait_ge(ws, g + 1)
                w_src = w[g][:].unsqueeze(1).to_broadcast((128, H, D))
                scalar.dma_start(out=out_v[g], in_=w_src).then_inc(st, 16)
            scalar.wait_ge(st, 48)

    nc.compile()
    return nc


def _build_tile(krep=1):
    """TileContext-scheduled equivalent (bitwise-identical outputs)."""
    import concourse.tile as tile
    from concourse import bacc, mybir

    f32 = mybir.dt.float32
    nc = bacc.Bacc("TRN2", target_bir_lowering=False, debug=False,
                   num_devices=NCORES)
    # v shard for batch b, viewed as (l, g, p, d): row l*256 + g*128 + p of v[b]
    v = nc.dram_tensor("v", [H, 2, 128, D], f32, kind="ExternalInput").ap()
    # out_v viewed as (g, p, h, d): row 16*(g*128+p) + h of V[b]
    out_v = nc.dram_tensor("out_v", [2, 128, H, D], f32,
                           kind="ExternalOutput").ap()
    # out_a: flat 4MB of the constant 1/256 -> host reshapes to (256,16,256)
    out_a = nc.dram_tensor("out_a", [4, 128, 2048], f32,
                           kind="ExternalOutput").ap()

    with tile.TileContext(nc) as tc:
        with tc.tile_pool(name="vin", bufs=4) as vin, \
             tc.tile_pool(name="work", bufs=2) as work, \
             tc.tile_pool(name="aconst", bufs=1) as apool:
            atile = apool.tile([128, 2048], f32)
            nc.vector.memset(atile[:], RECIP)
            for rep in range(krep):
                # All 4 input loads (2MB each) first, on the sync/SP HWDGE
                # ring, so the DMA engines never starve.
                loads = []
                for g in range(2):
                    for j in range(2):
                        t = vin.tile([128, 8 * D], f32, tag="vload")
                        nc.sync.dma_start(
                            t[:].rearrange("p (l d) -> p l d", l=8),
                            v[8 * j:8 * j + 8, g, :, :].rearrange(
                                "l p d -> p l d"),
                        )
                        loads.append(t)
                # One 4MB constant-A write via a stride-0 source AP
                # (scalar/ACT HWDGE ring, so writes don't block loads).
                a_src = atile[:].unsqueeze(1).to_broadcast((128, 4, 2048))
                nc.scalar.dma_start(out_a[:].rearrange("k p c -> p k c"),
                                    a_src)

                for g in range(2):
                    t0, t1 = loads[2 * g], loads[2 * g + 1]
                    s = work.tile([128, 8 * D], f32, tag="s")
                    nc.vector.tensor_add(s[:], t0[:], t1[:])
                    r4 = work.tile([128, 4 * D], f32, tag="r4")
                    nc.vector.tensor_add(r4[:], s[:, 0:4 * D], s[:, 4 * D:8 * D])
                    r2 = work.tile([128, 2 * D], f32, tag="r2")
                    nc.vector.tensor_add(r2[:], r4[:, 0:2 * D], r4[:, 2 * D:4 * D])
                    r1 = work.tile([128, D], f32, tag="r1")
                    nc.vector.tensor_add(r1[:], r2[:, 0:D], r2[:, D:2 * D])
                    w = work.tile([128, D], f32, tag="w")
                    nc.vector.tensor_scalar_mul(w[:], r1[:], RECIP)
                    # One 4MB V write per half: 16-way head broadcast via
                    # stride-0 source dim -> fully contiguous DRAM write.
                    w_src = w[:].unsqueeze(1).to_broadcast((128, H, D))
                    nc.scalar.dma_start(out_v[g], w_src)

    nc.compile()
    return nc


def _build_nc(krep=1):
    if os.environ.get("KERNEL_IMPL", "raw") == "tile":
        return _build_tile(krep)
    if krep != 1:  # krep unrolling only exists in the tile builder
        return _build_tile(krep)
    return _build_raw()


def _get_nc():
    if "nc" not in _CACHE:
        _CACHE["nc"] = _build_nc()
    return _CACHE["nc"]


def _run(v_full, trace=False):
    from concourse.bass_utils import run_bass_kernel_spmd

    nc = _get_nc()
    in_maps = [
        {"v": np.ascontiguousarray(
            v_full[b].reshape(H, 2, 128, D), dtype=np.float32)}
        for b in range(NCORES)
    ]
    try:
        res = run_bass_kernel_spmd(nc, in_maps, list(range(NCORES)),
                                   trace=trace)
    except Exception:
        # One retry for transient execution-path failures.
        import time as _time
        _time.sleep(2.0)
        res = run_bass_kernel_spmd(nc, in_maps, list(range(NCORES)),
                                   trace=trace)
    V = np.empty((B, L, D), dtype=np.float32)
    A = np.empty((B, N, H, N), dtype=np.float32)
    for b in range(NCORES):
        V[b] = res.results[b]["out_v"].reshape(L, D)
        A[b] = res.results[b]["out_a"].reshape(N, H, N)
    return (V, A), res


def kernel(q, v):
    v = np.asarray(v, dtype=np.float32)
    (V, A), _ = _run(v, trace=False)
    return V, A
